# revision 1
# baseline (speedup 1.0000x reference)
"""Trainium2 Bass kernel for nn_CAAN_78323023610440.

Reference computation (per batch b):
    q = x @ Wq.T + bq;  k = x @ Wk.T + bk;  v = x @ Wv.T + bv
    beta = softmax(q @ k.T / sqrt(D), axis=-1)
    final = (beta @ v) @ Ww.T + bw            # [B, N]

Algebraic restructuring (exact, modulo fp reassociation):
  *  q·k = x A x^T + r[n] + c[m] + const, with A = Wq^T Wk,
     r[n] = x[n]·(Wq^T bk) (row-constant -> drops out of softmax),
     c[m] = x[m]·(Wk^T bq) (key-side constant, kept).
  *  (beta @ v) @ Ww^T = beta @ (v @ Ww^T) = beta @ (x @ (Wv^T Ww^T) + bv·Ww)
     -> the whole V projection collapses into a per-key scalar wv[m].
  *  final[n] = sum_m exp(s[n,m]) * wv[m] / sum_m exp(s[n,m]) + bw
     (softmax max-subtraction skipped: logits are O(1) here, exp is safe
      in fp32 — both sums are formed unnormalized and divided at the end).

Sharding: 8 cores = 4 batches x 2 query-halves. Each core computes, for
its 1024 queries n and all 2048 keys m of its batch:
    TT[e, n]  = sum_f A[f, e] xT[f, n]            (phase 1)
    S^T[m, n] = sum_e xT[e, m] TT[e, n]           (phase 2, keys on partitions)
    P^T       = Exp(S^T/32 + c[m]/32)             (ScalarE, bias per partition)
    acc[0, n] = sum_m P^T[m, n] * wv[m]           (tiny PE matmul vs [wv, 1])
    acc[1, n] = sum_m P^T[m, n]
Host divides acc0/acc1 and adds bw. Key columns are passed to each core
local-half-first so all 8 cores run an identical program (SPMD).

MODE selects the PE dtype for the heavy matmuls (PSUM accumulation is
fp32 in both):
  "f32r": TF32-class single-pass fp32. Measured ~119 us / 9.7e-5 rel err.
  "bf16": bf16 operands. Measured ~115 us / 1.5e-3 rel err.
Both stream 1 moving column/cycle on the PE, so bf16's only win is the
halved input-DMA window; f32r is the default for its 15x accuracy margin.
"""

import numpy as np
from contextlib import ExitStack

import ml_dtypes

import concourse.tile as tile
from concourse import bacc, mybir
from concourse.bass_utils import run_bass_kernel_spmd

B = 4
N = 2048
D = 1024
P = 128
ET = D // P          # 8 contraction tiles over D
MT = N // P          # 16 key tiles
NQ = N // 2          # 1024 local queries per core
CHUNK = 512          # PSUM bank limit (512 fp32 outputs)
NCH = NQ // CHUNK    # 2 query chunks
SCALE = 0.03125      # 1/sqrt(D), exact
WARMUP_MM = 8        # dummy matmuls to lift the PE HAM clock-gate early
F32 = mybir.dt.float32
F32R = mybir.dt.float32r
BF16 = mybir.dt.bfloat16
EXP = mybir.ActivationFunctionType.Exp

MODE = "f32r"

_CACHE = {}


def _mm_dt():
    return BF16 if MODE == "bf16" else F32R


def _np_in_dt():
    return ml_dtypes.bfloat16 if MODE == "bf16" else np.float32


def _build():
    mdt = _mm_dt()
    nc = bacc.Bacc(
        "TRN2",
        target_bir_lowering=False,
        debug=False,
        enable_asserts=False,
        num_devices=8,
    )
    # Per-core inputs. xq = x[b, local half].T ; xk2 = x[b, other half].T
    # (keys ordered local-first so the program is core-independent).
    xq_d = nc.dram_tensor("xq", [D, NQ], mdt, kind="ExternalInput")
    xk2_d = nc.dram_tensor("xk2", [D, NQ], mdt, kind="ExternalInput")
    a_d = nc.dram_tensor("A", [D, D], mdt, kind="ExternalInput")
    cb_d = nc.dram_tensor("cb", [P, MT], F32, kind="ExternalInput")
    rv_d = nc.dram_tensor("rv", [P, MT, 2], mdt, kind="ExternalInput")
    out_d = nc.dram_tensor("out", [2, NQ], F32, kind="ExternalOutput")

    with tile.TileContext(nc) as tc, ExitStack() as ctx:
        const = ctx.enter_context(tc.tile_pool(name="const", bufs=1))
        ptp = ctx.enter_context(tc.tile_pool(name="pt", bufs=6))
        workp = ctx.enter_context(
            tc.tile_pool(name="psum_work", bufs=5, space="PSUM")
        )
        accp = ctx.enter_context(
            tc.tile_pool(name="psum_acc", bufs=1, space="PSUM")
        )
        wup = ctx.enter_context(
            tc.tile_pool(name="psum_wu", bufs=1, space="PSUM")
        )

        xq_sb = const.tile([P, ET, NQ], mdt)    # [p, f, n] : xT local cols
        xk2_sb = const.tile([P, ET, NQ], mdt)   # [p, f, n] : xT other cols
        a_sb = const.tile([P, ET, D], mdt)      # [p, f, e] : A tiles
        tt_sb = const.tile([P, ET, NQ], mdt)    # [p, e, n] : TT tiles
        cb_sb = const.tile([P, MT], F32)        # exp bias c[m]/32
        rv_sb = const.tile([P, MT, 2], mdt)     # [wv[m], 1] per key tile
        out_sb = const.tile([2, NQ], F32)
        wu_sb = const.tile([P, CHUNK], BF16)    # warmup operand (garbage ok)
        wu_sink = const.tile([P, 1], F32)

        # PE warm-up: keep TensorE busy from t~0 so the HAM clock-gate
        # lifts to 8/8 before the real matmuls start (they are DMA-gated),
        # and fill the DMA-paced holes of the first TT block below.
        # Operand contents are irrelevant.
        nc.gpsimd.memset(wu_sb[:], 0.0)
        wu_ps = wup.tile([P, CHUNK], F32)
        # per-f filler count topping up the real matmuls one (A[f], xq[f])
        # tile-pair arrival enables in the first block; >2 measured worse
        wpf = 2
        n_wu = WARMUP_MM + wpf * ET
        wu_iter = iter(range(n_wu))

        def warm(k):
            for _ in range(k):
                w = next(wu_iter, None)
                if w is None:
                    return
                nc.tensor.matmul(
                    wu_ps[:],
                    wu_sb[:, :P],
                    wu_sb[:],
                    start=(w == 0),
                    stop=(w == n_wu - 1),
                )

        warm(WARMUP_MM)

        # Input DMAs. Phase-1 block 0 (e 0-2) needs only A columns 0:384,
        # so those stream first alongside xq — this shrinks the critical
        # bytes gating the first matmul block from 8MB to 5.5MB. The rest
        # of A arrives while block 0 computes; xk2 only gates key tiles
        # 8-15 of phase 2, so it streams last.
        E0 = 3 * P
        for f in range(ET):
            nc.sync.dma_start(a_sb[:, f, :E0], a_d[f * P:(f + 1) * P, :E0])
            nc.sync.dma_start(xq_sb[:, f, :], xq_d[f * P:(f + 1) * P, :])
        nc.sync.dma_start(cb_sb[:], cb_d[:])
        nc.sync.dma_start(rv_sb[:], rv_d[:])
        # Keep these as per-f DMA instructions: separate instructions fan
        # out across parallel HW DMA queues (consolidating them into one
        # strided DMA measured ~6us slower end-to-end).
        for f in range(ET):
            nc.sync.dma_start(a_sb[:, f, E0:], a_d[f * P:(f + 1) * P, E0:])
        for f in range(ET):
            nc.sync.dma_start(xk2_sb[:, f, :], xk2_d[f * P:(f + 1) * P, :])

        # Phase 1: TT[e, n] = sum_f A[f, e-cols]^T . xT[f, n]
        # e-blocks of 3 keep 6 PSUM accumulation groups open so each
        # arriving (A[f], xq[f]) DMA pair feeds 6 matmuls (less PE
        # starvation while inputs stream in). The acc-pool banks are idle
        # during phase 1, so two groups per block borrow them.
        BLOCKS = [(0, 3), (3, 3), (6, 2)]
        for eb, (e0, blk) in enumerate(BLOCKS):
            pss = []
            for el in range(blk):
                row = []
                for j in range(NCH):
                    k = el * NCH + j
                    if k < 4:
                        row.append(workp.tile(
                            [P, CHUNK], F32, name=f"tt_ps_{eb}_{el}_{j}", tag="ps"))
                    else:
                        row.append(accp.tile(
                            [P, CHUNK], F32, name=f"tt_acc_{eb}_{el}_{j}",
                            tag=f"acc{j}"))
                pss.append(row)
            for f in range(ET):
                for el in range(blk):
                    e = e0 + el
                    for j in range(NCH):
                        nc.tensor.matmul(
                            pss[el][j][:],
                            a_sb[:, f, e * P:(e + 1) * P],
                            xq_sb[:, f, j * CHUNK:(j + 1) * CHUNK],
                            start=(f == 0),
                            stop=(f == ET - 1),
                        )
                if eb == 0:
                    # absorb the DMA-arrival pacing of the first block
                    warm(wpf)
            for el in range(blk):
                e = e0 + el
                for j in range(NCH):
                    nc.vector.tensor_copy(
                        tt_sb[:, e, j * CHUNK:(j + 1) * CHUNK], pss[el][j][:]
                    )
            if eb == 0:
                warm(100)  # flush any leftover warmups
                nc.vector.tensor_copy(wu_sink[:], wu_ps[:, :1])

        # Phase 2: per key tile t: S^T, exp, and the [wv, 1] reduction.
        # The reduction matmul for tile t is issued one tile late so it
        # never stalls PE waiting on ScalarE's exp of tile t. (fp32-family
        # matmuls reject out base_partition != 0, so the two accumulators
        # get separate banks.)
        accs = [accp.tile([2, CHUNK], F32, name=f"acc{j}", tag=f"acc{j}")
                for j in range(NCH)]

        def reduce_mm(t, pt, j):
            nc.tensor.matmul(
                accs[j][:],
                rv_sb[:, t, :],
                pt[:],
                start=(t == 0),
                stop=(t == MT - 1),
            )

        prev = None
        for t in range(MT):
            xsrc = xq_sb if t < ET else xk2_sb
            off = (t % ET) * P
            pts_t = []
            for j in range(NCH):
                ps = workp.tile([P, CHUNK], F32, name=f"st_ps_{t}_{j}", tag="ps")
                for e in range(ET):
                    nc.tensor.matmul(
                        ps[:],
                        xsrc[:, e, off:off + P],
                        tt_sb[:, e, j * CHUNK:(j + 1) * CHUNK],
                        start=(e == 0),
                        stop=(e == ET - 1),
                    )
                pt = ptp.tile([P, CHUNK], mdt, name=f"pt_{t}_{j}", tag="pt")
                nc.scalar.activation(
                    pt[:], ps[:], EXP, bias=cb_sb[:, t:t + 1], scale=SCALE
                )
                pts_t.append(pt)
                # issue the t-1 reduction for this j between the two ST
                # groups of t so the two tiny matmuls never sit adjacent
                if prev is not None:
                    reduce_mm(t - 1, prev[j], j)
            prev = pts_t
        # epilogue: j0's last reduction can run while ScalarE still
        # computes exp of the last j1 tile
        reduce_mm(MT - 1, prev[0], 0)
        reduce_mm(MT - 1, prev[1], 1)

        # Phase 3: move the two [2, 512] accumulators out on different
        # engines so the copies overlap.
        nc.scalar.copy(out_sb[:, 0:CHUNK], accs[0][:])
        nc.vector.tensor_copy(out_sb[:, CHUNK:NQ], accs[1][:])
        nc.sync.dma_start(out_d[:], out_sb[:])

    nc.compile()
    return nc


def _get_nc():
    if "nc" not in _CACHE:
        _CACHE["nc"] = _build()
    return _CACHE["nc"]


def _prep(x, Wq, bq, Wk, bk, Wv, bv, Ww, bw):
    """Host-side sharding + weight folding -> per-core input maps."""
    x = np.asarray(x, dtype=np.float32)
    Wq = np.asarray(Wq, dtype=np.float32)
    bq = np.asarray(bq, dtype=np.float32)
    Wk = np.asarray(Wk, dtype=np.float32)
    bk = np.asarray(bk, dtype=np.float32)
    Wv = np.asarray(Wv, dtype=np.float32)
    bv = np.asarray(bv, dtype=np.float32)
    Ww = np.asarray(Ww, dtype=np.float32)
    idt = _np_in_dt()

    # Host-side weight folding (cheap: one 1024^3 sgemm + matvecs).
    A = np.ascontiguousarray(Wq.T @ Wk)             # [f, e]
    g = Wk.T @ bq                                   # key-side logit constant
    wv_eff = Wv.T @ Ww[0]                           # collapsed V @ Ww^T
    cvw = float(bv @ Ww[0])

    c_all = (x @ g) * SCALE                         # [B, N] exp bias (pre-scaled)
    wv_all = x @ wv_eff + cvw                       # [B, N]

    A = A.astype(idt)
    in_maps = []
    for core in range(8):
        b, h = divmod(core, 2)
        lo = np.arange(h * NQ, (h + 1) * NQ)
        hi = np.arange((1 - h) * NQ, (2 - h) * NQ)
        order = np.concatenate([lo, hi])            # keys: local half first
        cb = np.ascontiguousarray(c_all[b][order].reshape(MT, P).T)
        rv = np.ascontiguousarray(
            np.stack(
                [wv_all[b][order].reshape(MT, P).T.astype(idt),
                 np.ones((P, MT), idt)],
                axis=-1,
            )
        )
        in_maps.append(
            {
                "xq": np.ascontiguousarray(x[b, lo].T.astype(idt)),
                "xk2": np.ascontiguousarray(x[b, hi].T.astype(idt)),
                "A": A,
                "cb": cb,
                "rv": rv,
            }
        )
    return in_maps


def _gather(res, bw):
    bw = np.asarray(bw, dtype=np.float32)
    final = np.empty((B, N), dtype=np.float32)
    for core in range(8):
        b, h = divmod(core, 2)
        o = res.results[core]["out"]
        final[b, h * NQ:(h + 1) * NQ] = o[0] / o[1] + bw[0]
    return final


def kernel(x, Wq, bq, Wk, bk, Wv, bv, Ww, bw):
    nc = _get_nc()
    in_maps = _prep(x, Wq, bq, Wk, bk, Wv, bv, Ww, bw)
    res = run_bass_kernel_spmd(nc, in_maps, core_ids=list(range(8)))
    return _gather(res, bw)


def run_profiled(inputs, trace_cores=(0,)):
    """Run once with NTFF profiling; returns BassKernelResults."""
    nc = _get_nc()
    in_maps = _prep(**inputs)
    res = run_bass_kernel_spmd(
        nc, in_maps, core_ids=list(range(8)), trace=True,
        trace_cores=list(trace_cores),
    )
    return res



# revision 8
# speedup vs baseline: 1.5035x; 1.5035x over previous
"""Trainium2 Bass kernel for nn_CAAN_78323023610440.

Reference computation (per batch b):
    q = x @ Wq.T + bq;  k = x @ Wk.T + bk;  v = x @ Wv.T + bv
    beta = softmax(q @ k.T / sqrt(D), axis=-1)
    final = (beta @ v) @ Ww.T + bw            # [B, N]

Algebraic restructuring (exact, modulo fp reassociation):
  *  q·k = x A x^T + r[n] + c[m] + const, with A = Wq^T Wk,
     r[n] = x[n]·(Wq^T bk) (row-constant -> drops out of softmax),
     c[m] = x[m]·(Wk^T bq) (key-side constant, kept).
  *  (beta @ v) @ Ww^T = beta @ (v @ Ww^T) = beta @ (x @ (Wv^T Ww^T) + bv·Ww)
     -> the whole V projection collapses into a per-key scalar wv[m].
  *  final[n] = sum_m exp(s[n,m]) * wv[m] / sum_m exp(s[n,m]) + bw
     (softmax max-subtraction skipped: logits are O(1) here, exp is safe
      in fp32 — both sums are formed unnormalized and divided at the end).

Sharding: 8 cores = 4 batches x 2 query-halves. Each core computes, for
its 1024 queries n and all 2048 keys m of its batch:
    TT[e, n]  = sum_f A[f, e] xT[f, n]            (phase 1)
    S^T[m, n] = sum_e xT[e, m] TT[e, n]           (phase 2, keys on partitions)
    P^T       = Exp(S^T/32 + c[m]/32)             (ScalarE, bias per partition)
    acc[0, n] = sum_m P^T[m, n] * wv[m]           (tiny PE matmul vs [wv, 1])
    acc[1, n] = sum_m P^T[m, n]
Host divides acc0/acc1 and adds bw. Key columns are passed to each core
local-half-first so all 8 cores run an identical program (SPMD).

MODE selects the PE dtype for the heavy matmuls (PSUM accumulation is
fp32 in all):
  "f32r": TF32-class single-pass fp32. Measured ~119 us / 9.7e-5 rel err.
  "bf16": bf16 operands. Measured ~115 us / 1.5e-3 rel err.
  "fp8":  fp8e4 (e4m3) operands with MatmulPerfMode.DoubleRow: each
          matmul packs TWO adjacent 128-deep contraction tiles
          (stationary [128,2,128], moving [128,2,512]), which the PE
          streams at 2 contraction-rows/cycle. A is pre-scaled by 32 on
          the host so its entries (std ~0.01) sit in e4m3's normal
          range; the exp activation scale compensates (1/1024).
"""

import numpy as np
from contextlib import ExitStack

import ml_dtypes

import concourse.tile as tile
from concourse import bacc, mybir
from concourse.bass_utils import run_bass_kernel_spmd

B = 4
N = 2048
D = 1024
P = 128
ET = D // P          # 8 contraction tiles over D
MT = N // P          # 16 key tiles
NQ = N // 2          # 1024 local queries per core
CHUNK = 512          # PSUM bank limit (512 fp32 outputs)
NCH = NQ // CHUNK    # 2 query chunks
SCALE = 0.03125      # 1/sqrt(D), exact
WARMUP_MM = 8        # dummy matmuls to lift the PE HAM clock-gate early
F32 = mybir.dt.float32
F32R = mybir.dt.float32r
BF16 = mybir.dt.bfloat16
FP8 = mybir.dt.float8e4
EXP = mybir.ActivationFunctionType.Exp
DR = mybir.MatmulPerfMode.DoubleRow

MODE = "fp8"
A_SCALE = 32.0       # fp8 mode: host multiplies A by this, exp scale divides

_CACHE = {}


def _mm_dt():
    return {"bf16": BF16, "fp8": FP8}.get(MODE, F32R)


def _np_in_dt():
    if MODE == "bf16":
        return ml_dtypes.bfloat16
    if MODE == "fp8":
        return ml_dtypes.float8_e4m3
    return np.float32


def _build():
    mdt = _mm_dt()
    # pt (exp output) + rv (reduction weights) dtype: bf16 in fp8 mode —
    # quantizing the softmax weights to fp8 costs ~5e-3 rel err for only
    # ~7us of PE time, so the reduction stays bf16.
    pdt = BF16 if MODE == "fp8" else mdt
    ascale = SCALE / A_SCALE if MODE == "fp8" else SCALE
    nsteps = ET // 2 if MODE == "fp8" else ET  # contraction steps / output
    pmode = DR if MODE == "fp8" else None
    nc = bacc.Bacc(
        "TRN2",
        target_bir_lowering=False,
        debug=False,
        enable_asserts=False,
        num_devices=8,
    )
    # Per-core inputs. xq = x[b, local half].T ; xk2 = x[b, other half].T
    # (keys ordered local-first so the program is core-independent).
    xq_d = nc.dram_tensor("xq", [D, NQ], mdt, kind="ExternalInput")
    xk2_d = nc.dram_tensor("xk2", [D, NQ], mdt, kind="ExternalInput")
    a_d = nc.dram_tensor("A", [D, D], mdt, kind="ExternalInput")
    cb_d = nc.dram_tensor("cb", [P, MT], F32, kind="ExternalInput")
    rv_d = nc.dram_tensor("rv", [P, MT, 2], pdt, kind="ExternalInput")
    out_d = nc.dram_tensor("out", [2, NQ], F32, kind="ExternalOutput")

    with tile.TileContext(nc) as tc, ExitStack() as ctx:
        const = ctx.enter_context(tc.tile_pool(name="const", bufs=1))
        ptp = ctx.enter_context(tc.tile_pool(name="pt", bufs=6))
        workp = ctx.enter_context(
            tc.tile_pool(name="psum_work", bufs=5, space="PSUM")
        )
        accp = ctx.enter_context(
            tc.tile_pool(name="psum_acc", bufs=1, space="PSUM")
        )
        wup = ctx.enter_context(
            tc.tile_pool(name="psum_wu", bufs=1, space="PSUM")
        )

        xq_sb = const.tile([P, ET, NQ], mdt)    # [p, f, n] : xT local cols
        xk2_sb = const.tile([P, ET, NQ], mdt)   # [p, f, n] : xT other cols
        a_sb = const.tile([P, ET, D], mdt)      # [p, f, e] : A tiles
        tt_sb = const.tile([P, ET, NQ], mdt)    # [p, e, n] : TT tiles
        cb_sb = const.tile([P, MT], F32)        # exp bias c[m]/32
        rv_sb = const.tile([P, MT, 2], pdt)     # [wv[m], 1] per key tile
        out_sb = const.tile([2, NQ], F32)
        wu_sb = const.tile([P, CHUNK], BF16)    # warmup operand (garbage ok)
        wu_sink = const.tile([P, 1], F32)

        # PE warm-up: keep TensorE busy from t~0 so the HAM clock-gate
        # lifts to 8/8 before the real matmuls start (they are DMA-gated),
        # and fill the DMA-paced holes of the first TT block below.
        # Operand contents are irrelevant.
        nc.gpsimd.memset(wu_sb[:], 0.0)
        wu_ps = wup.tile([P, CHUNK], F32)
        # per-step filler count topping up the real matmuls one (A, xq)
        # tile-pair arrival enables in the first block; >2 measured worse
        wpf = 2
        n_wu = WARMUP_MM + wpf * nsteps
        wu_iter = iter(range(n_wu))

        def warm(k):
            for _ in range(k):
                w = next(wu_iter, None)
                if w is None:
                    return
                nc.tensor.matmul(
                    wu_ps[:],
                    wu_sb[:, :P],
                    wu_sb[:],
                    start=(w == 0),
                    stop=(w == n_wu - 1),
                )

        warm(WARMUP_MM)

        # Input DMAs. Phase-1 block 0 (e 0-2) needs only A columns 0:384,
        # so those stream first alongside xq — this shrinks the critical
        # bytes gating the first matmul block from 8MB to 5.5MB. The rest
        # of A arrives while block 0 computes; xk2 only gates key tiles
        # 8-15 of phase 2, so it streams last.
        E0 = 3 * P
        for f in range(ET):
            nc.sync.dma_start(a_sb[:, f, :E0], a_d[f * P:(f + 1) * P, :E0])
            nc.sync.dma_start(xq_sb[:, f, :], xq_d[f * P:(f + 1) * P, :])
        nc.sync.dma_start(cb_sb[:], cb_d[:])
        nc.sync.dma_start(rv_sb[:], rv_d[:])
        # Keep these as per-f DMA instructions: separate instructions fan
        # out across parallel HW DMA queues (consolidating them into one
        # strided DMA measured ~6us slower end-to-end).
        for f in range(ET):
            nc.sync.dma_start(a_sb[:, f, E0:], a_d[f * P:(f + 1) * P, E0:])
        for f in range(ET):
            nc.sync.dma_start(xk2_sb[:, f, :], xk2_d[f * P:(f + 1) * P, :])

        # Phase 1: TT[e, n] = sum_f A[f, e-cols]^T . xT[f, n]
        # e-blocks of 3 keep 6 PSUM accumulation groups open so each
        # arriving (A[f], xq[f]) DMA pair feeds 6 matmuls (less PE
        # starvation while inputs stream in). The acc-pool banks are idle
        # during phase 1, so two groups per block borrow them.
        BLOCKS = [(0, 3), (3, 3), (6, 2)]
        for eb, (e0, blk) in enumerate(BLOCKS):
            pss = []
            for el in range(blk):
                row = []
                for j in range(NCH):
                    k = el * NCH + j
                    if k < 4:
                        row.append(workp.tile(
                            [P, CHUNK], F32, name=f"tt_ps_{eb}_{el}_{j}", tag="ps"))
                    else:
                        row.append(accp.tile(
                            [P, CHUNK], F32, name=f"tt_acc_{eb}_{el}_{j}",
                            tag=f"acc{j}"))
                pss.append(row)
            for ci in range(nsteps):
                for el in range(blk):
                    e = e0 + el
                    for j in range(NCH):
                        if MODE == "fp8":
                            lhs = a_sb[:, 2 * ci:2 * ci + 2, e * P:(e + 1) * P]
                            rhs = xq_sb[:, 2 * ci:2 * ci + 2,
                                        j * CHUNK:(j + 1) * CHUNK]
                        else:
                            lhs = a_sb[:, ci, e * P:(e + 1) * P]
                            rhs = xq_sb[:, ci, j * CHUNK:(j + 1) * CHUNK]
                        nc.tensor.matmul(
                            pss[el][j][:],
                            lhs,
                            rhs,
                            start=(ci == 0),
                            stop=(ci == nsteps - 1),
                            perf_mode=pmode,
                        )
                if eb == 0:
                    # absorb the DMA-arrival pacing of the first block
                    warm(wpf)
            for el in range(blk):
                e = e0 + el
                for j in range(NCH):
                    nc.vector.tensor_copy(
                        tt_sb[:, e, j * CHUNK:(j + 1) * CHUNK], pss[el][j][:]
                    )
            if eb == 0:
                warm(100)  # flush any leftover warmups
                nc.vector.tensor_copy(wu_sink[:], wu_ps[:, :1])

        # Phase 2: per key tile t: S^T, exp, and the [wv, 1] reduction.
        # The reduction matmul for tile t is issued one tile late so it
        # never stalls PE waiting on ScalarE's exp of tile t. (fp32-family
        # matmuls reject out base_partition != 0, so the two accumulators
        # get separate banks.)
        accs = [accp.tile([2, CHUNK], F32, name=f"acc{j}", tag=f"acc{j}")
                for j in range(NCH)]

        def reduce_mm(t, pt, j):
            nc.tensor.matmul(
                accs[j][:],
                rv_sb[:, t, :],
                pt[:],
                start=(t == 0),
                stop=(t == MT - 1),
            )

        prev = None
        for t in range(MT):
            xsrc = xq_sb if t < ET else xk2_sb
            off = (t % ET) * P
            pts_t = []
            for j in range(NCH):
                ps = workp.tile([P, CHUNK], F32, name=f"st_ps_{t}_{j}", tag="ps")
                for ci in range(nsteps):
                    if MODE == "fp8":
                        lhs = xsrc[:, 2 * ci:2 * ci + 2, off:off + P]
                        rhs = tt_sb[:, 2 * ci:2 * ci + 2,
                                    j * CHUNK:(j + 1) * CHUNK]
                    else:
                        lhs = xsrc[:, ci, off:off + P]
                        rhs = tt_sb[:, ci, j * CHUNK:(j + 1) * CHUNK]
                    nc.tensor.matmul(
                        ps[:],
                        lhs,
                        rhs,
                        start=(ci == 0),
                        stop=(ci == nsteps - 1),
                        perf_mode=pmode,
                    )
                pt = ptp.tile([P, CHUNK], pdt, name=f"pt_{t}_{j}", tag="pt")
                nc.scalar.activation(
                    pt[:], ps[:], EXP, bias=cb_sb[:, t:t + 1], scale=ascale
                )
                pts_t.append(pt)
                # issue the t-1 reduction for this j between the two ST
                # groups of t so the two tiny matmuls never sit adjacent
                if prev is not None:
                    reduce_mm(t - 1, prev[j], j)
            prev = pts_t
        # epilogue: j0's last reduction can run while ScalarE still
        # computes exp of the last j1 tile
        reduce_mm(MT - 1, prev[0], 0)
        reduce_mm(MT - 1, prev[1], 1)

        # Phase 3: move the two [2, 512] accumulators out on different
        # engines so the copies overlap.
        nc.scalar.copy(out_sb[:, 0:CHUNK], accs[0][:])
        nc.vector.tensor_copy(out_sb[:, CHUNK:NQ], accs[1][:])
        nc.sync.dma_start(out_d[:], out_sb[:])

    nc.compile()
    return nc


def _get_nc():
    if "nc" not in _CACHE:
        _CACHE["nc"] = _build()
    return _CACHE["nc"]


def _prep(x, Wq, bq, Wk, bk, Wv, bv, Ww, bw):
    """Host-side sharding + weight folding -> per-core input maps."""
    x = np.asarray(x, dtype=np.float32)
    Wq = np.asarray(Wq, dtype=np.float32)
    bq = np.asarray(bq, dtype=np.float32)
    Wk = np.asarray(Wk, dtype=np.float32)
    bk = np.asarray(bk, dtype=np.float32)
    Wv = np.asarray(Wv, dtype=np.float32)
    bv = np.asarray(bv, dtype=np.float32)
    Ww = np.asarray(Ww, dtype=np.float32)
    idt = _np_in_dt()

    # Host-side weight folding (cheap: one 1024^3 sgemm + matvecs).
    A = np.ascontiguousarray(Wq.T @ Wk)             # [f, e]
    g = Wk.T @ bq                                   # key-side logit constant
    wv_eff = Wv.T @ Ww[0]                           # collapsed V @ Ww^T
    cvw = float(bv @ Ww[0])

    c_all = (x @ g) * SCALE                         # [B, N] exp bias (pre-scaled)
    wv_all = x @ wv_eff + cvw                       # [B, N]

    if MODE == "fp8":
        # lift A (std ~0.01) out of e4m3's subnormal range; the exp's
        # scale parameter compensates (SCALE / A_SCALE)
        A = A * A_SCALE
    A = A.astype(idt)
    pnp = ml_dtypes.bfloat16 if MODE == "fp8" else idt
    in_maps = []
    for core in range(8):
        b, h = divmod(core, 2)
        lo = np.arange(h * NQ, (h + 1) * NQ)
        hi = np.arange((1 - h) * NQ, (2 - h) * NQ)
        order = np.concatenate([lo, hi])            # keys: local half first
        cb = np.ascontiguousarray(c_all[b][order].reshape(MT, P).T)
        rv = np.ascontiguousarray(
            np.stack(
                [wv_all[b][order].reshape(MT, P).T.astype(pnp),
                 np.ones((P, MT), pnp)],
                axis=-1,
            )
        )
        in_maps.append(
            {
                "xq": np.ascontiguousarray(x[b, lo].T.astype(idt)),
                "xk2": np.ascontiguousarray(x[b, hi].T.astype(idt)),
                "A": A,
                "cb": cb,
                "rv": rv,
            }
        )
    return in_maps


def _gather(res, bw):
    bw = np.asarray(bw, dtype=np.float32)
    final = np.empty((B, N), dtype=np.float32)
    for core in range(8):
        b, h = divmod(core, 2)
        o = res.results[core]["out"]
        final[b, h * NQ:(h + 1) * NQ] = o[0] / o[1] + bw[0]
    return final


def kernel(x, Wq, bq, Wk, bk, Wv, bv, Ww, bw):
    nc = _get_nc()
    in_maps = _prep(x, Wq, bq, Wk, bk, Wv, bv, Ww, bw)
    res = run_bass_kernel_spmd(nc, in_maps, core_ids=list(range(8)))
    return _gather(res, bw)


def run_profiled(inputs, trace_cores=(0,)):
    """Run once with NTFF profiling; returns BassKernelResults."""
    nc = _get_nc()
    in_maps = _prep(**inputs)
    res = run_bass_kernel_spmd(
        nc, in_maps, core_ids=list(range(8)), trace=True,
        trace_cores=list(trace_cores),
    )
    return res



# revision 11
# speedup vs baseline: 1.7450x; 1.1606x over previous
"""Trainium2 Bass kernel for nn_CAAN_78323023610440.

Reference computation (per batch b):
    q = x @ Wq.T + bq;  k = x @ Wk.T + bk;  v = x @ Wv.T + bv
    beta = softmax(q @ k.T / sqrt(D), axis=-1)
    final = (beta @ v) @ Ww.T + bw            # [B, N]

Algebraic restructuring (exact, modulo fp reassociation):
  *  q.k = x A x^T + r[n] + c[m] + const, with A = Wq^T Wk,
     r[n] = x[n].(Wq^T bk) (row-constant -> drops out of softmax),
     c[m] = x[m].(Wk^T bq) (key-side constant, kept). c is LINEAR in
     x[m], so it folds into the TT operand: c[m] = sum_e g[e] x[m,e]
     with g = Wk^T bq -> add g to every column of TT (see below).
  *  (beta @ v) @ Ww^T = beta @ (x @ (Wv^T Ww^T) + bv.Ww)
     -> the whole V projection collapses into a per-key scalar wv[m].
  *  final[n] = sum_m exp(s[n,m]) wv[m] / sum_m exp(s[n,m]) + bw
     (softmax max-subtraction skipped: logits are O(1) here, exp is safe
      in fp32 — both sums are formed unnormalized and divided at the end).

Sharding: 8 cores = 4 batches x 2 query-halves. Each core computes, for
its 1024 queries n and all 2048 keys m of its batch (keys ordered
local-half-first so all 8 cores run an identical SPMD program):
    TT[e, n]  = sum_f A[f, e] xT[f, n] + g[e]     (phase 1 + DVE add)
    S[n, m]   = sum_e TT[e, n] xT[e, m]           (phase 2, QUERIES on
                                                   partitions, keys free)
    pt        = Exp(S/32/32)                      (ScalarE; accum_out
                                                   gives den = sum_m pt)
    num[n]    = sum_m pt[n, m] wv[m]              (DVE tensor_tensor_reduce
                                                   vs wv replicated row)
Host divides num/den and adds bw.

The orientation (queries on PSUM partitions) makes both softmax sums
FREE-dim reductions, so no PE matmuls are spent on them — the old
key-partition layout burned ~20% of PE time on [128,2]-stationary
reduction matmuls at 0.8% PE efficiency.

All heavy matmuls run in fp8e4 (e4m3) with MatmulPerfMode.DoubleRow:
each instruction contracts TWO 128-deep tiles (stationary [128,2,128],
moving [128,2,512]) at ~1 column/cycle — the 157 TF/s fp8 peak. A is
pre-scaled by 32 on the host so its entries (std ~0.01) sit in e4m3's
normal range; the exp scale compensates (1/1024). Measured numerics:
~1.4e-2 max rel err vs the 2e-2 gate (dominated by fp8 quantization of
x and TT; pt/wv stay bf16, accumulation fp32).

Inputs are host-prearranged into partition-contiguous SBUF images so
the whole input set streams in 8 large DMAs (the per-slice DMA fan-out
of the v1 kernel serialized ~35 descriptor pushes at ~600ns each on the
Sync engine and starved phase 1).
"""

import numpy as np
from contextlib import ExitStack

import ml_dtypes

import concourse.tile as tile
from concourse import bacc, mybir
from concourse.bass_utils import run_bass_kernel_spmd

B = 4
N = 2048
D = 1024
P = 128
ET = D // P          # 8 contraction tiles over D
NQ = N // 2          # 1024 local queries per core
NT = NQ // P         # 8 query tiles
CHUNK = 512          # PSUM bank limit (512 fp32 outputs)
KC = N // CHUNK      # 4 key chunks
NCH = NQ // CHUNK    # 2 query chunks (phase 1 moving)
NP = ET // 2         # 4 DoubleRow contraction pairs
E0T = 3              # e-tiles in phase-1 block 0 (A split a0/a1)
SCALE = 0.03125      # 1/sqrt(D), exact
A_SCALE = 32.0       # host multiplies A,g by this; exp scale divides
WARMUP_MM = 8        # dummy matmuls to lift the PE HAM clock-gate early
F32 = mybir.dt.float32
BF16 = mybir.dt.bfloat16
FP8 = mybir.dt.float8e4
EXP = mybir.ActivationFunctionType.Exp
DR = mybir.MatmulPerfMode.DoubleRow
ADD = mybir.AluOpType.add
MULT = mybir.AluOpType.mult
AXX = mybir.AxisListType.X

_CACHE = {}


def _build():
    nc = bacc.Bacc(
        "TRN2",
        target_bir_lowering=False,
        debug=False,
        enable_asserts=False,
        num_devices=8,
    )
    # Host-prearranged, partition-contiguous inputs (see _prep):
    #   x  [p, f, m] = x[key m, f*128+p]      (keys local-half-first)
    #   a0 [p, f, e] = 32*A[f*128+p, e]        e in [0, 384)
    #   a1 [p, f, e] = 32*A[f*128+p, 384+e]    e in [0, 640)
    #   gt [p, et]   = 32*g[et*128+p]
    #   wv [p, m]    = wv[m]  (replicated rows, for the DVE reduce)
    x_d = nc.dram_tensor("x", [P, ET, N], FP8, kind="ExternalInput")
    a0_d = nc.dram_tensor("a0", [P, ET, E0T * P], FP8, kind="ExternalInput")
    a1_d = nc.dram_tensor("a1", [P, ET, (ET - E0T) * P], FP8,
                          kind="ExternalInput")
    gt_d = nc.dram_tensor("gt", [P, ET], F32, kind="ExternalInput")
    wv_d = nc.dram_tensor("wv", [P, N], BF16, kind="ExternalInput")
    out_d = nc.dram_tensor("out", [P, NT, 2], F32, kind="ExternalOutput")

    with tile.TileContext(nc) as tc, ExitStack() as ctx:
        const = ctx.enter_context(tc.tile_pool(name="const", bufs=1))
        ptp = ctx.enter_context(tc.tile_pool(name="pt", bufs=4))
        scrp = ctx.enter_context(tc.tile_pool(name="scr", bufs=2))
        workp = ctx.enter_context(
            tc.tile_pool(name="psum_work", bufs=6, space="PSUM")
        )
        wup = ctx.enter_context(
            tc.tile_pool(name="psum_wu", bufs=1, space="PSUM")
        )

        x_sb = const.tile([P, ET, N], FP8)      # xT, queries 0:NQ / keys all
        a0_sb = const.tile([P, ET, E0T * P], FP8)
        a1_sb = const.tile([P, ET, (ET - E0T) * P], FP8)
        tt_sb = const.tile([P, ET, NQ], FP8)    # TT' = 32(A^T xq^T + g)
        gt_sb = const.tile([P, ET], F32)
        wv_sb = const.tile([P, N], BF16)
        den_sb = const.tile([P, NT, KC], F32)   # per-chunk denominators
        num_sb = const.tile([P, NT, KC], F32)   # per-chunk numerators
        out_sb = const.tile([P, NT, 2], F32)
        wu_sb = const.tile([P, CHUNK], BF16)    # warmup operand (garbage ok)
        wu_sink = const.tile([P, 1], F32)

        # PE warm-up: keep TensorE busy from t~0 so the HAM clock-gate
        # lifts to 8/8 before the real matmuls start (they are DMA-gated),
        # and fill the DMA-paced holes of the first TT block below.
        nc.gpsimd.memset(wu_sb[:], 0.0)
        wu_ps = wup.tile([P, CHUNK], F32)
        wpf = 2
        n_wu = WARMUP_MM + wpf * NP
        wu_iter = iter(range(n_wu))

        def warm(k):
            for _ in range(k):
                w = next(wu_iter, None)
                if w is None:
                    return
                nc.tensor.matmul(
                    wu_ps[:],
                    wu_sb[:, :P],
                    wu_sb[:],
                    start=(w == 0),
                    stop=(w == n_wu - 1),
                )

        warm(WARMUP_MM)

        # Input DMAs: few and large (contiguous per partition). a0 + x
        # pair 0 gate the first phase-1 block; the rest streams behind.
        nc.sync.dma_start(a0_sb[:], a0_d[:])
        for i in range(NP):
            nc.sync.dma_start(x_sb[:, 2 * i:2 * i + 2, :],
                              x_d[:, 2 * i:2 * i + 2, :])
            if i == 0:
                nc.sync.dma_start(gt_sb[:], gt_d[:])
            if i == 1:
                nc.sync.dma_start(a1_sb[:], a1_d[:])
        nc.sync.dma_start(wv_sb[:], wv_d[:])

        def a_slc(ci, e):
            if e < E0T:
                return a0_sb[:, 2 * ci:2 * ci + 2, e * P:(e + 1) * P]
            eo = e - E0T
            return a1_sb[:, 2 * ci:2 * ci + 2, eo * P:(eo + 1) * P]

        # Phase 1: TT[e, n] = sum_f A[f, e-cols]^T . xT[f, n] over the
        # local-query columns. e-blocks of 3 keep 6 PSUM groups open so
        # each arriving (A, x) pair DMA feeds 6 matmuls.
        BLOCKS = [(0, 3), (3, 3), (6, 2)]
        for eb, (e0, blk) in enumerate(BLOCKS):
            pss = [
                [workp.tile([P, CHUNK], F32, name=f"tt_ps_{eb}_{el}_{j}",
                            tag="ps") for j in range(NCH)]
                for el in range(blk)
            ]
            for ci in range(NP):
                for el in range(blk):
                    e = e0 + el
                    for j in range(NCH):
                        nc.tensor.matmul(
                            pss[el][j][:],
                            a_slc(ci, e),
                            x_sb[:, 2 * ci:2 * ci + 2,
                                 j * CHUNK:(j + 1) * CHUNK],
                            start=(ci == 0),
                            stop=(ci == NP - 1),
                            perf_mode=DR,
                        )
                if eb == 0:
                    # absorb the DMA-arrival pacing of the first block
                    warm(wpf)
            for el in range(blk):
                e = e0 + el
                for j in range(NCH):
                    # fused add of the key-side constant g + fp8 cast
                    nc.vector.tensor_scalar(
                        tt_sb[:, e, j * CHUNK:(j + 1) * CHUNK],
                        pss[el][j][:],
                        gt_sb[:, e:e + 1],
                        None,
                        ADD,
                    )
            if eb == 0:
                warm(100)  # flush any leftover warmups
                nc.vector.tensor_copy(wu_sink[:], wu_ps[:, :1])

        # Phase 2: per (query-tile nt, key-chunk ch): S tile, exp with
        # free-dim accumulation (denominator), DVE weighted reduce
        # (numerator).
        for nt in range(NT):
            for ch in range(KC):
                ps = workp.tile([P, CHUNK], F32, name=f"s_ps_{nt}_{ch}",
                                tag="ps")
                for ci in range(NP):
                    nc.tensor.matmul(
                        ps[:],
                        tt_sb[:, 2 * ci:2 * ci + 2, nt * P:(nt + 1) * P],
                        x_sb[:, 2 * ci:2 * ci + 2,
                             ch * CHUNK:(ch + 1) * CHUNK],
                        start=(ci == 0),
                        stop=(ci == NP - 1),
                        perf_mode=DR,
                    )
                pt = ptp.tile([P, CHUNK], BF16, name=f"pt_{nt}_{ch}",
                              tag="pt")
                nc.scalar.activation(
                    pt[:], ps[:], EXP, scale=SCALE / A_SCALE,
                    accum_out=den_sb[:, nt, ch:ch + 1],
                )
                # (tensor_tensor_reduce would fuse these, but that opcode
                # hard-faults this runtime's exec unit — two DVE passes)
                scr = scrp.tile([P, CHUNK], BF16, name=f"scr_{nt}_{ch}",
                                tag="scr")
                nc.vector.tensor_tensor(
                    scr[:], pt[:], wv_sb[:, ch * CHUNK:(ch + 1) * CHUNK],
                    MULT,
                )
                nc.vector.tensor_reduce(
                    num_sb[:, nt, ch:ch + 1], scr[:], AXX, ADD,
                )

        # Phase 3: fold the KC per-chunk partials and ship (num, den).
        nc.vector.tensor_reduce(out_sb[:, :, 0:1], num_sb[:], AXX, ADD)
        nc.vector.tensor_reduce(out_sb[:, :, 1:2], den_sb[:], AXX, ADD)
        nc.sync.dma_start(out_d[:], out_sb[:])

    nc.compile()
    return nc


def _get_nc():
    if "nc" not in _CACHE:
        _CACHE["nc"] = _build()
    return _CACHE["nc"]


def _prep(x, Wq, bq, Wk, bk, Wv, bv, Ww, bw):
    """Host-side sharding + weight folding -> per-core input maps."""
    x = np.asarray(x, dtype=np.float32)
    Wq = np.asarray(Wq, dtype=np.float32)
    bq = np.asarray(bq, dtype=np.float32)
    Wk = np.asarray(Wk, dtype=np.float32)
    bk = np.asarray(bk, dtype=np.float32)
    Wv = np.asarray(Wv, dtype=np.float32)
    bv = np.asarray(bv, dtype=np.float32)
    Ww = np.asarray(Ww, dtype=np.float32)
    f8 = ml_dtypes.float8_e4m3
    bf = ml_dtypes.bfloat16

    # Host-side weight folding (cheap: one 1024^3 sgemm + matvecs).
    A = (Wq.T @ Wk) * A_SCALE                       # [f, e], fp8-range
    g = (Wk.T @ bq) * A_SCALE                       # key-side logit constant
    wv_eff = Wv.T @ Ww[0]                           # collapsed V @ Ww^T
    cvw = float(bv @ Ww[0])
    wv_all = x @ wv_eff + cvw                       # [B, N]

    a_im = np.ascontiguousarray(
        A.reshape(ET, P, D).transpose(1, 0, 2))     # [p, f, e]
    a0 = np.ascontiguousarray(a_im[:, :, :E0T * P]).astype(f8)
    a1 = np.ascontiguousarray(a_im[:, :, E0T * P:]).astype(f8)
    gt = np.ascontiguousarray(g.reshape(ET, P).T)   # [p, et] f32

    in_maps = []
    for core in range(8):
        b, h = divmod(core, 2)
        lo = np.arange(h * NQ, (h + 1) * NQ)
        hi = np.arange((1 - h) * NQ, (2 - h) * NQ)
        order = np.concatenate([lo, hi])            # keys: local half first
        xim = np.ascontiguousarray(
            x[b][order].T.reshape(ET, P, N).transpose(1, 0, 2)
        ).astype(f8)                                # [p, f, m]
        wvr = np.ascontiguousarray(
            np.broadcast_to(wv_all[b][order][None, :].astype(bf), (P, N))
        )
        in_maps.append({"x": xim, "a0": a0, "a1": a1, "gt": gt, "wv": wvr})
    return in_maps


def _gather(res, bw):
    bw = np.asarray(bw, dtype=np.float32)
    final = np.empty((B, N), dtype=np.float32)
    for core in range(8):
        b, h = divmod(core, 2)
        o = res.results[core]["out"]                # [p, nt, 2]
        num = o[:, :, 0].T.reshape(NQ)              # n = nt*128 + p
        den = o[:, :, 1].T.reshape(NQ)
        final[b, h * NQ:(h + 1) * NQ] = num / den + bw[0]
    return final


def kernel(x, Wq, bq, Wk, bk, Wv, bv, Ww, bw):
    nc = _get_nc()
    in_maps = _prep(x, Wq, bq, Wk, bk, Wv, bv, Ww, bw)
    res = run_bass_kernel_spmd(nc, in_maps, core_ids=list(range(8)))
    return _gather(res, bw)


def run_profiled(inputs, trace_cores=(0,)):
    """Run once with NTFF profiling; returns BassKernelResults."""
    nc = _get_nc()
    in_maps = _prep(**inputs)
    res = run_bass_kernel_spmd(
        nc, in_maps, core_ids=list(range(8)), trace=True,
        trace_cores=list(trace_cores),
    )
    return res
